# revision 10
# baseline (speedup 1.0000x reference)
"""NetVLAD Trainium2 kernel (v2).

Problem: nn_NetVLAD (N=32, C=512, H=W=32, K=64), data-parallel over batch
across 8 NeuronCores (4 images per core). Params are tiny and replicated.

Math (per image, P = H*W = 1024 pixels):
  xhat[:,p]  = x[:,p] / max(||x[:,p]||, eps)
  logits     = conv_w @ xhat + b            # [K, P]
  sa         = softmax_k(logits)
  vlad[k,c]  = sum_p sa[k,p]*xhat[c,p] - (sum_p sa[k,p]) * cent[k,c]
  out        = l2norm_c(vlad) / sqrt(K)     # global l2 norm == sqrt(K) exactly

v2 design:
  - logits0 = W @ x on raw fp32 x: f32r matmuls, Wt stationary, N=512.
  - logits^T via 8 PE transposes (f32r, exact); scaled by the per-pixel
    1/||x|| (broadcast-materialized) on VectorE, one big Exp on ScalarE.
    conv_b folded in multiplicatively via eb = exp(b); softmax max-subtract
    dropped analytically (logits are O(5)).
  - x^T in BF16: GpSimd casts x -> bf16, 32 PE transposes (bf16, 1 cyc/row),
    ScalarE evacuates PSUM->SBUF; per-pixel ssq via VectorE
    tensor_tensor_reduce on the bf16 x^T.
  - 1/sqrt via Ln+Exp (one ACT table set, no Sqrt-set thrash) + one
    Newton-rsqrt refinement for the logit-critical r.
  - vlad = sa2^T.T @ x^T in bf16 (fp32 PSUM accumulate); softmax sums via
    fp32 matmul Eb^T @ (1/s); centroid term via lhsT = diag(-sasum) f32r.
  - intra-norm scale 1/(8*||vlad_k||) folds the exact global norm.
"""

import numpy as np

try:
    import concourse.bass as bass
except ImportError:  # pragma: no cover
    import sys

    for _p in ("/opt/trn_rl_repo", "/root/.axon_site/_ro/trn_rl_repo"):
        sys.path.insert(0, _p)
    import concourse.bass as bass

import concourse.bacc as bacc
import concourse.mybir as mybir
import concourse.tile as tile
from concourse.bass_utils import run_bass_kernel_spmd
from concourse.masks import make_identity

F32 = mybir.dt.float32
F32R = mybir.dt.float32r
BF16 = mybir.dt.bfloat16
AF = mybir.ActivationFunctionType
ALU = mybir.AluOpType

import os

N, C, HW, K = 32, 512, 1024, 64

# v2 feature flags (bisection)
F_BF16XT = os.environ.get("NV_BF16XT", "1") == "1"   # bf16 x^T pipeline
F_BATCHEXP = os.environ.get("NV_BATCHEXP", "1") == "1"  # R_full + one big exp
F_LNEXP = os.environ.get("NV_LNEXP", "1") == "1"     # rsqrt via Ln+Exp
F_ACTOUT = os.environ.get("NV_ACTOUT", "1") == "1"   # vout via ACT Copy scale
NCORES = 8
NPER = N // NCORES  # 4 images per core
NCHUNK = C // 128  # 4 c-chunks
NPT = HW // 128  # 8 pixel tiles


def netvlad_tile_kernel(ctx, tc, out_d, x_d, wt_d, eb_d, cent_d):
    nc = tc.nc

    singles = ctx.enter_context(tc.tile_pool(name="singles", bufs=1))
    px = ctx.enter_context(tc.tile_pool(name="px", bufs=2))
    pxb = ctx.enter_context(tc.tile_pool(name="pxb", bufs=2))
    pxt = ctx.enter_context(tc.tile_pool(name="pxt", bufs=2))
    plog = ctx.enter_context(tc.tile_pool(name="plog", bufs=2))
    psoft = ctx.enter_context(tc.tile_pool(name="psoft", bufs=2))
    small = ctx.enter_context(tc.tile_pool(name="small", bufs=2))
    junkp = ctx.enter_context(tc.tile_pool(name="junk", bufs=1))
    pout = ctx.enter_context(tc.tile_pool(name="pout", bufs=2))

    ps_l_pool = ctx.enter_context(tc.tile_pool(name="psl", bufs=1, space="PSUM"))
    ps_xt_pool = ctx.enter_context(tc.tile_pool(name="psxt", bufs=3, space="PSUM"))
    ps_lt_pool = ctx.enter_context(tc.tile_pool(name="pslt", bufs=1, space="PSUM"))
    ps_v_pool = ctx.enter_context(tc.tile_pool(name="psv", bufs=1, space="PSUM"))
    ps_s_pool = ctx.enter_context(tc.tile_pool(name="pss", bufs=1, space="PSUM"))

    # ---- persistent setup -------------------------------------------------
    wt_s = singles.tile([128, NCHUNK, K], F32R)  # conv_w.T as 4 [128, 64] chunks
    nc.sync.dma_start(
        out=wt_s, in_=wt_d.rearrange("(j p) k -> p j k", p=128).bitcast(F32R)
    )

    cent_s = singles.tile([K, C], F32R)
    nc.sync.dma_start(out=cent_s, in_=cent_d[:, :].bitcast(F32R))

    # exp(conv_b), broadcast to all 128 partitions and repeated per p-tile
    eb_s = singles.tile([128, NPT, K], F32)
    eb_ap = eb_d[:]
    nc.gpsimd.dma_start(
        out=eb_s,
        in_=bass.AP(tensor=eb_ap.tensor, offset=eb_ap.offset,
                    ap=[[0, 128], [0, NPT], [1, K]]),
    )

    ident_f = singles.tile([128, 128], F32)
    make_identity(nc, ident_f)
    ident = singles.tile([128, 128], F32R)
    nc.vector.tensor_copy(ident, ident_f)
    identB = singles.tile([128, 128], BF16)
    nc.vector.tensor_copy(identB, ident_f)

    for n in range(NPER):
        # ---- load x[n] : [C, HW] as [128, 4, 1024]; bf16 shadow copy -----
        X = px.tile([128, NCHUNK, HW], F32R)
        nc.sync.dma_start(
            out=X, in_=x_d[n].rearrange("(j p) m -> p j m", p=128).bitcast(F32R)
        )
        if F_BF16XT:
            Xb = pxb.tile([128, NCHUNK, HW], BF16)
            for j in range(NCHUNK):
                nc.gpsimd.tensor_copy(Xb[:, j, :], X[:, j, :].bitcast(F32))

        # ---- matmul1: logits0 = conv_w @ x -> PSUM [64, 1024], f32r ------
        ps_l = ps_l_pool.tile([K, HW], F32)
        for h in range(2):
            sl = slice(512 * h, 512 * (h + 1))
            for j in range(NCHUNK):
                nc.tensor.matmul(
                    ps_l[:, sl],
                    wt_s[:, j, :],
                    X[:, j, sl],
                    start=(j == 0),
                    stop=(j == NCHUNK - 1),
                )

        # evacuate logits0 to SBUF (ScalarE)
        L = plog.tile([K, HW], F32R)
        nc.scalar.copy(L, ps_l)

        # ---- x^T (bf16): PE transposes; evac on ScalarE; ssq on VectorE --
        xt_dt = BF16 if F_BF16XT else F32R
        XT = pxt.tile([128, NPT, 512], xt_dt)
        ssq = small.tile([128, NPT], F32)
        junkb = junkp.tile([128, 512], BF16, tag="junkb")
        junk = junkp.tile([128, 512], F32, tag="junk")
        for t in range(NPT):
            ps_xt = ps_xt_pool.tile([128, 512], xt_dt)
            for j in range(NCHUNK):
                nc.tensor.transpose(
                    ps_xt[:, 128 * j:128 * (j + 1)],
                    (Xb if F_BF16XT else X)[:, j, 128 * t:128 * (t + 1)],
                    identB if F_BF16XT else ident,
                )
            if F_BF16XT:
                # evac split ACT/DVE; ssq: 5 on ACT (Square+accum from PSUM),
                # 3 on DVE (mul+reduce from SBUF after evac)
                if t % 2 == 0:
                    nc.scalar.copy(XT[:, t, :], ps_xt)
                else:
                    nc.vector.tensor_copy(XT[:, t, :], ps_xt)
                if t < 5:
                    nc.scalar.activation(
                        out=junk, in_=ps_xt.bitcast(BF16), func=AF.Square,
                        accum_out=ssq[:, t:t + 1],
                    )
                else:
                    nc.vector.tensor_mul(junkb, XT[:, t, :], XT[:, t, :])
                    nc.vector.reduce_sum(
                        ssq[:, t:t + 1], junkb, axis=mybir.AxisListType.X
                    )
            else:
                nc.scalar.activation(
                    out=junk, in_=ps_xt.bitcast(F32), func=AF.Square,
                    accum_out=ssq[:, t:t + 1],
                )
                if t % 2 == 0:
                    nc.vector.tensor_copy(XT[:, t, :], ps_xt)
                else:
                    nc.scalar.copy(XT[:, t, :], ps_xt)

        # ---- r = 1/sqrt(max(ssq, tiny))  [128, 8] ------------------------
        ssqc = small.tile([128, NPT], F32, tag="ssqc")
        nc.vector.tensor_scalar_max(ssqc, ssq, 1e-24)
        r = small.tile([128, NPT], F32)
        if F_LNEXP:
            lnr = small.tile([128, NPT], F32, tag="lnr")
            nc.scalar.activation(out=lnr, in_=ssqc, func=AF.Ln)
            nc.scalar.activation(out=r, in_=lnr, func=AF.Exp, scale=-0.5)
        else:
            nrm = small.tile([128, NPT], F32, tag="nrm")
            nc.scalar.activation(out=nrm, in_=ssqc, func=AF.Sqrt)
            nc.vector.reciprocal(r, nrm)
        # one Newton-rsqrt step: r <- r * (1.5 - 0.5 * ssqc * r^2)
        t1 = small.tile([128, NPT], F32, tag="newt")
        nc.vector.tensor_mul(t1, r, r)
        nc.vector.tensor_mul(t1, t1, ssqc)
        nc.vector.tensor_scalar(
            out=t1, in0=t1, scalar1=-0.5, scalar2=1.5,
            op0=ALU.mult, op1=ALU.add,
        )
        nc.vector.tensor_mul(r, r, t1)
        if F_BATCHEXP:
            # broadcast r along k: R_full[p, t, k] = r[p, t]
            R_full = psoft.tile([128, NPT, K], F32, tag="rfull")
            r_rep = bass.AP(tensor=r.tensor, offset=r.offset,
                            ap=[r.ap[0], [1, NPT], [0, K]])
            nc.gpsimd.tensor_copy(R_full, r_rep)

        # ---- logits^T via PE transpose; E = exp(logits^T * r) ------------
        ps_lt = ps_lt_pool.tile([128, NPT, K], F32R)
        for t in range(NPT):
            nc.tensor.transpose(
                ps_lt[:, t, :],
                L[:, 128 * t:128 * (t + 1)],
                ident[:K, :K],
            )
        E = psoft.tile([128, NPT, K], F32)
        if F_BATCHEXP:
            LTs = psoft.tile([128, NPT, K], F32, tag="lts")
            nc.vector.tensor_mul(LTs, ps_lt.bitcast(F32), R_full)
            nc.scalar.activation(out=E, in_=LTs, func=AF.Exp)
        else:
            for t in range(NPT):
                nc.scalar.activation(
                    out=E[:, t, :], in_=ps_lt[:, t, :].bitcast(F32),
                    func=AF.Exp, scale=r[:, t:t + 1],
                )

        # ---- softmax: Eb = E * exp(b); s = sum_k Eb ----------------------
        Eb = psoft.tile([128, NPT, K], F32)
        if F_BATCHEXP:
            nc.gpsimd.tensor_mul(Eb, E, eb_s)
        else:
            nc.vector.tensor_mul(Eb, E, eb_s)
        s = small.tile([128, NPT], F32)
        nc.vector.reduce_sum(s, Eb, axis=mybir.AxisListType.X)
        sinv = small.tile([128, NPT], F32)
        nc.vector.reciprocal(sinv, s)
        t2 = small.tile([128, NPT], F32)
        nc.vector.tensor_mul(t2, r, sinv)

        sa2 = psoft.tile([128, NPT, K], xt_dt)  # softmax * r
        for t in range(NPT):
            nc.vector.tensor_scalar_mul(sa2[:, t, :], Eb[:, t, :], t2[:, t:t + 1])

        # ---- vlad = sa2^T.T @ x^T (bf16); sasum = Eb^T @ (1/s) (fp32) ----
        ps_v = ps_v_pool.tile([K, C], F32)
        ps_s = ps_s_pool.tile([K, 1], F32)
        for t in range(NPT):
            nc.tensor.matmul(
                ps_v, sa2[:, t, :], XT[:, t, :],
                start=(t == 0), stop=False,
            )
            nc.tensor.matmul(
                ps_s, Eb[:, t, :], sinv[:, t:t + 1],
                start=(t == 0), stop=(t == NPT - 1),
            )

        # centroid correction: vlad -= sasum * cent  (lhsT = diag(-sasum))
        negsum = small.tile([K, 1], F32, tag="negsum")
        nc.vector.tensor_scalar_mul(negsum, ps_s, -1.0)
        diag = small.tile([K, K], F32R, tag="diag")
        nc.vector.tensor_scalar_mul(diag, ident_f[:K, :K], negsum)
        nc.tensor.matmul(ps_v, diag, cent_s, start=False, stop=True)

        # ---- intra-norm + folded global norm -----------------------------
        junk64 = junkp.tile([K, 512], F32, tag="junk64")
        ssqv = small.tile([K, 1], F32, tag="ssqv")
        nc.scalar.activation(out=junk64, in_=ps_v, func=AF.Square,
                             accum_out=ssqv)
        # invn = 1/(8*||vlad_k||) = exp(-0.5*ln(64*ssqv))
        invn = small.tile([K, 1], F32, tag="invn")
        if F_LNEXP:
            lnv = small.tile([K, 1], F32, tag="lnv")
            nc.scalar.activation(out=lnv, in_=ssqv, func=AF.Ln, scale=64.0)
            nc.scalar.activation(out=invn, in_=lnv, func=AF.Exp, scale=-0.5)
        else:
            nv = small.tile([K, 1], F32, tag="nv")
            nc.scalar.activation(out=nv, in_=ssqv, func=AF.Sqrt, scale=64.0)
            nc.vector.reciprocal(invn, nv)

        vout = pout.tile([K, C], F32)
        if F_ACTOUT:
            nc.scalar.activation(out=vout, in_=ps_v, func=AF.Copy, scale=invn)
        else:
            nc.vector.tensor_scalar_mul(vout, ps_v, invn)
        nc.sync.dma_start(out=out_d[n], in_=vout)


def build_nc():
    nc = bacc.Bacc("TRN2", target_bir_lowering=False, debug=False)
    x_d = nc.dram_tensor("x", [NPER, C, HW], F32, kind="ExternalInput")
    wt_d = nc.dram_tensor("wt", [C, K], F32, kind="ExternalInput")
    eb_d = nc.dram_tensor("eb", [K], F32, kind="ExternalInput")
    cent_d = nc.dram_tensor("cent", [K, C], F32, kind="ExternalInput")
    out_d = nc.dram_tensor("out", [NPER, K, C], F32, kind="ExternalOutput")

    from contextlib import ExitStack

    with tile.TileContext(nc) as tc:
        with ExitStack() as ctx:
            netvlad_tile_kernel(ctx, tc, out_d, x_d, wt_d, eb_d, cent_d)
    nc.compile()
    return nc


_NC = None


def _get_nc():
    global _NC
    if _NC is None:
        _NC = build_nc()
    return _NC


def make_in_maps(x, conv_w, conv_b, centroids):
    wt = np.ascontiguousarray(conv_w.astype(np.float32).T)  # [C, K]
    eb = np.exp(conv_b.astype(np.float64)).astype(np.float32)  # [K]
    cent = np.ascontiguousarray(centroids.astype(np.float32))
    xs = np.ascontiguousarray(x.astype(np.float32).reshape(NCORES, NPER, C, HW))
    return [
        {"x": xs[c], "wt": wt, "eb": eb, "cent": cent} for c in range(NCORES)
    ]


def kernel(x, conv_w, conv_b, centroids, **_ignored):
    nc = _get_nc()
    in_maps = make_in_maps(x, conv_w, conv_b, centroids)
    res = run_bass_kernel_spmd(nc, in_maps, core_ids=list(range(NCORES)))
    outs = [res.results[c]["out"].reshape(NPER, K * C) for c in range(NCORES)]
    return np.concatenate(outs, axis=0).astype(np.float32)


if __name__ == "__main__":
    nc = build_nc()
    print("built ok:", len(nc.inst_map), "instructions")


# revision 12
# speedup vs baseline: 1.2569x; 1.2569x over previous
"""NetVLAD Trainium2 kernel (v2).

Problem: nn_NetVLAD (N=32, C=512, H=W=32, K=64), data-parallel over batch
across 8 NeuronCores (4 images per core). Params are tiny and replicated.

Math (per image, P = H*W = 1024 pixels):
  xhat[:,p]  = x[:,p] / max(||x[:,p]||, eps)
  logits     = conv_w @ xhat + b            # [K, P]
  sa         = softmax_k(logits)
  vlad[k,c]  = sum_p sa[k,p]*xhat[c,p] - (sum_p sa[k,p]) * cent[k,c]
  out        = l2norm_c(vlad) / sqrt(K)     # global l2 norm == sqrt(K) exactly

v2 design:
  - logits0 = W @ x on raw fp32 x: f32r matmuls, Wt stationary, N=512.
  - logits^T via 8 PE transposes (f32r, exact); scaled by the per-pixel
    1/||x|| (broadcast-materialized) on VectorE, one big Exp on ScalarE.
    conv_b folded in multiplicatively via eb = exp(b); softmax max-subtract
    dropped analytically (logits are O(5)).
  - x^T in BF16: GpSimd casts x -> bf16, 32 PE transposes (bf16, 1 cyc/row),
    ScalarE evacuates PSUM->SBUF; per-pixel ssq via VectorE
    tensor_tensor_reduce on the bf16 x^T.
  - 1/sqrt via Ln+Exp (one ACT table set, no Sqrt-set thrash) + one
    Newton-rsqrt refinement for the logit-critical r.
  - vlad = sa2^T.T @ x^T in bf16 (fp32 PSUM accumulate); softmax sums via
    fp32 matmul Eb^T @ (1/s); centroid term via lhsT = diag(-sasum) f32r.
  - intra-norm scale 1/(8*||vlad_k||) folds the exact global norm.
"""

import numpy as np

try:
    import concourse.bass as bass
except ImportError:  # pragma: no cover
    import sys

    for _p in ("/opt/trn_rl_repo", "/root/.axon_site/_ro/trn_rl_repo"):
        sys.path.insert(0, _p)
    import concourse.bass as bass

import concourse.bacc as bacc
import concourse.mybir as mybir
import concourse.tile as tile
from concourse.bass_utils import run_bass_kernel_spmd
from concourse.masks import make_identity

F32 = mybir.dt.float32
F32R = mybir.dt.float32r
BF16 = mybir.dt.bfloat16
AF = mybir.ActivationFunctionType
ALU = mybir.AluOpType

import os

N, C, HW, K = 32, 512, 1024, 64

# v2 feature flags (bisection)
F_BF16XT = os.environ.get("NV_BF16XT", "1") == "1"   # bf16 x^T pipeline
F_BATCHEXP = os.environ.get("NV_BATCHEXP", "1") == "1"  # R_full + one big exp
F_LNEXP = os.environ.get("NV_LNEXP", "0") == "1"     # rsqrt via Ln+Exp
F_ACTOUT = os.environ.get("NV_ACTOUT", "1") == "1"   # vout via ACT Copy scale
NCORES = 8
NPER = N // NCORES  # 4 images per core
NCHUNK = C // 128  # 4 c-chunks
NPT = HW // 128  # 8 pixel tiles


def netvlad_tile_kernel(ctx, tc, out_d, x_d, wt_d, eb_d, cent_d):
    nc = tc.nc

    singles = ctx.enter_context(tc.tile_pool(name="singles", bufs=1))
    px = ctx.enter_context(tc.tile_pool(name="px", bufs=2))
    pxb = ctx.enter_context(tc.tile_pool(name="pxb", bufs=2))
    pxt = ctx.enter_context(tc.tile_pool(name="pxt", bufs=2))
    plog = ctx.enter_context(tc.tile_pool(name="plog", bufs=2))
    psoft = ctx.enter_context(tc.tile_pool(name="psoft", bufs=2))
    small = ctx.enter_context(tc.tile_pool(name="small", bufs=2))
    junkp = ctx.enter_context(tc.tile_pool(name="junk", bufs=1))
    pout = ctx.enter_context(tc.tile_pool(name="pout", bufs=2))

    ps_l_pool = ctx.enter_context(tc.tile_pool(name="psl", bufs=1, space="PSUM"))
    ps_xt_pool = ctx.enter_context(tc.tile_pool(name="psxt", bufs=3, space="PSUM"))
    ps_lt_pool = ctx.enter_context(tc.tile_pool(name="pslt", bufs=1, space="PSUM"))
    ps_v_pool = ctx.enter_context(tc.tile_pool(name="psv", bufs=1, space="PSUM"))
    ps_s_pool = ctx.enter_context(tc.tile_pool(name="pss", bufs=1, space="PSUM"))

    # ---- persistent setup -------------------------------------------------
    wt_s = singles.tile([128, NCHUNK, K], F32R)  # conv_w.T as 4 [128, 64] chunks
    nc.sync.dma_start(
        out=wt_s, in_=wt_d.rearrange("(j p) k -> p j k", p=128).bitcast(F32R)
    )

    cent_s = singles.tile([K, C], F32R)
    nc.sync.dma_start(out=cent_s, in_=cent_d[:, :].bitcast(F32R))

    # exp(conv_b), broadcast to all 128 partitions and repeated per p-tile
    eb_s = singles.tile([128, NPT, K], F32)
    eb_ap = eb_d[:]
    nc.gpsimd.dma_start(
        out=eb_s,
        in_=bass.AP(tensor=eb_ap.tensor, offset=eb_ap.offset,
                    ap=[[0, 128], [0, NPT], [1, K]]),
    )

    ident_f = singles.tile([128, 128], F32)
    make_identity(nc, ident_f)
    ident = singles.tile([128, 128], F32R)
    nc.vector.tensor_copy(ident, ident_f)
    identB = singles.tile([128, 128], BF16)
    nc.vector.tensor_copy(identB, ident_f)

    for n in range(NPER):
        # ---- load x[n] : [C, HW] as [128, 4, 1024]; bf16 shadow copy -----
        X = px.tile([128, NCHUNK, HW], F32R)
        nc.sync.dma_start(
            out=X, in_=x_d[n].rearrange("(j p) m -> p j m", p=128).bitcast(F32R)
        )
        if F_BF16XT:
            Xb = pxb.tile([128, NCHUNK, HW], BF16)
            for j in range(NCHUNK):
                nc.vector.tensor_copy(Xb[:, j, :], X[:, j, :].bitcast(F32))

        # ---- matmul1: logits0 = conv_w @ x -> PSUM [64, 1024], f32r ------
        ps_l = ps_l_pool.tile([K, HW], F32)
        for h in range(2):
            sl = slice(512 * h, 512 * (h + 1))
            for j in range(NCHUNK):
                nc.tensor.matmul(
                    ps_l[:, sl],
                    wt_s[:, j, :],
                    X[:, j, sl],
                    start=(j == 0),
                    stop=(j == NCHUNK - 1),
                )

        # evacuate logits0 to SBUF (ScalarE)
        L = plog.tile([K, HW], F32R)
        nc.scalar.copy(L, ps_l)

        # ---- x^T (bf16): PE transposes; evac on ScalarE; ssq on VectorE --
        xt_dt = BF16 if F_BF16XT else F32R
        XT = pxt.tile([128, NPT, 512], xt_dt)
        ssq = small.tile([128, NPT], F32)
        junkb = junkp.tile([128, 512], BF16, tag="junkb")
        junk = junkp.tile([128, 512], F32, tag="junk")
        for t in range(NPT):
            ps_xt = ps_xt_pool.tile([128, 512], xt_dt)
            for j in range(NCHUNK):
                nc.tensor.transpose(
                    ps_xt[:, 128 * j:128 * (j + 1)],
                    (Xb if F_BF16XT else X)[:, j, 128 * t:128 * (t + 1)],
                    identB if F_BF16XT else ident,
                )
            if F_BF16XT:
                # evac split ACT/DVE; ssq: 5 on ACT (Square+accum from PSUM),
                # 3 on DVE (mul+reduce from SBUF after evac)
                if t % 2 == 0:
                    nc.scalar.copy(XT[:, t, :], ps_xt)
                else:
                    nc.vector.tensor_copy(XT[:, t, :], ps_xt)
                if t < 5:
                    nc.scalar.activation(
                        out=junk, in_=ps_xt.bitcast(BF16), func=AF.Square,
                        accum_out=ssq[:, t:t + 1],
                    )
                else:
                    nc.vector.tensor_mul(junkb, XT[:, t, :], XT[:, t, :])
                    nc.vector.reduce_sum(
                        ssq[:, t:t + 1], junkb, axis=mybir.AxisListType.X
                    )
            else:
                nc.scalar.activation(
                    out=junk, in_=ps_xt.bitcast(F32), func=AF.Square,
                    accum_out=ssq[:, t:t + 1],
                )
                if t % 2 == 0:
                    nc.vector.tensor_copy(XT[:, t, :], ps_xt)
                else:
                    nc.scalar.copy(XT[:, t, :], ps_xt)

        # ---- r = 1/sqrt(max(ssq, tiny))  [128, 8] ------------------------
        ssqc = small.tile([128, NPT], F32, tag="ssqc")
        nc.vector.tensor_scalar_max(ssqc, ssq, 1e-24)
        r = small.tile([128, NPT], F32)
        if F_LNEXP:
            nc.scalar.activation(out=r, in_=ssqc, func=AF.Abs_reciprocal_sqrt)
        else:
            nrm = small.tile([128, NPT], F32, tag="nrm")
            nc.scalar.activation(out=nrm, in_=ssqc, func=AF.Sqrt)
            nc.vector.reciprocal(r, nrm)
        # one Newton-rsqrt step: r <- r * (1.5 - 0.5 * ssqc * r^2)
        t1 = small.tile([128, NPT], F32, tag="newt")
        nc.vector.tensor_mul(t1, r, r)
        nc.vector.tensor_mul(t1, t1, ssqc)
        nc.vector.tensor_scalar(
            out=t1, in0=t1, scalar1=-0.5, scalar2=1.5,
            op0=ALU.mult, op1=ALU.add,
        )
        nc.vector.tensor_mul(r, r, t1)
        if F_BATCHEXP:
            # broadcast r along k: R_full[p, t, k] = r[p, t]
            R_full = psoft.tile([128, NPT, K], F32, tag="rfull")
            r_rep = bass.AP(tensor=r.tensor, offset=r.offset,
                            ap=[r.ap[0], [1, NPT], [0, K]])
            nc.gpsimd.tensor_copy(R_full, r_rep)

        # ---- logits^T via PE transpose; E = exp(logits^T * r) ------------
        ps_lt = ps_lt_pool.tile([128, NPT, K], F32R)
        for t in range(NPT):
            nc.tensor.transpose(
                ps_lt[:, t, :],
                L[:, 128 * t:128 * (t + 1)],
                ident[:K, :K],
            )
        E = psoft.tile([128, NPT, K], F32)
        if F_BATCHEXP:
            LTs = psoft.tile([128, NPT, K], F32, tag="lts")
            nc.vector.tensor_mul(LTs, ps_lt.bitcast(F32), R_full)
            nc.scalar.activation(out=E, in_=LTs, func=AF.Exp)
        else:
            for t in range(NPT):
                nc.scalar.activation(
                    out=E[:, t, :], in_=ps_lt[:, t, :].bitcast(F32),
                    func=AF.Exp, scale=r[:, t:t + 1],
                )

        # ---- softmax: Eb = E * exp(b); s = sum_k Eb ----------------------
        Eb = psoft.tile([128, NPT, K], F32)
        if F_BATCHEXP:
            nc.gpsimd.tensor_mul(Eb, E, eb_s)
        else:
            nc.vector.tensor_mul(Eb, E, eb_s)
        s = small.tile([128, NPT], F32)
        nc.vector.reduce_sum(s, Eb, axis=mybir.AxisListType.X)
        sinv = small.tile([128, NPT], F32)
        nc.vector.reciprocal(sinv, s)
        t2 = small.tile([128, NPT], F32)
        nc.vector.tensor_mul(t2, r, sinv)
        T2_full = psoft.tile([128, NPT, K], F32, tag="t2full")
        t2_rep = bass.AP(tensor=t2.tensor, offset=t2.offset,
                         ap=[t2.ap[0], [1, NPT], [0, K]])
        nc.gpsimd.tensor_copy(T2_full, t2_rep)

        sa2 = psoft.tile([128, NPT, K], xt_dt)  # softmax * r
        nc.vector.tensor_mul(sa2, Eb, T2_full)

        # ---- vlad = sa2^T.T @ x^T (bf16); sasum = Eb^T @ (1/s) (fp32) ----
        ps_v = ps_v_pool.tile([K, C], F32)
        ps_s = ps_s_pool.tile([K, 1], F32)
        for t in range(NPT):
            nc.tensor.matmul(
                ps_v, sa2[:, t, :], XT[:, t, :],
                start=(t == 0), stop=False,
            )
            nc.tensor.matmul(
                ps_s, Eb[:, t, :], sinv[:, t:t + 1],
                start=(t == 0), stop=(t == NPT - 1),
            )

        # centroid correction: vlad -= sasum * cent  (lhsT = diag(-sasum))
        negsum = small.tile([K, 1], F32, tag="negsum")
        nc.vector.tensor_scalar_mul(negsum, ps_s, -1.0)
        diag = small.tile([K, K], F32R, tag="diag")
        nc.vector.tensor_scalar_mul(diag, ident_f[:K, :K], negsum)
        nc.tensor.matmul(ps_v, diag, cent_s, start=False, stop=True)

        # ---- intra-norm + folded global norm -----------------------------
        junk64 = junkp.tile([K, 512], F32, tag="junk64")
        ssqv = small.tile([K, 1], F32, tag="ssqv")
        nc.scalar.activation(out=junk64, in_=ps_v, func=AF.Square,
                             accum_out=ssqv)
        # invn = 1/(8*||vlad_k||) = exp(-0.5*ln(64*ssqv))
        invn = small.tile([K, 1], F32, tag="invn")
        if F_LNEXP:
            nc.scalar.activation(out=invn, in_=ssqv,
                                 func=AF.Abs_reciprocal_sqrt, scale=64.0)
        else:
            nv = small.tile([K, 1], F32, tag="nv")
            nc.scalar.activation(out=nv, in_=ssqv, func=AF.Sqrt, scale=64.0)
            nc.vector.reciprocal(invn, nv)

        vout = pout.tile([K, C], F32)
        if F_ACTOUT:
            nc.scalar.activation(out=vout, in_=ps_v, func=AF.Copy, scale=invn)
        else:
            nc.vector.tensor_scalar_mul(vout, ps_v, invn)
        nc.sync.dma_start(out=out_d[n], in_=vout)


def build_nc():
    nc = bacc.Bacc("TRN2", target_bir_lowering=False, debug=False)
    x_d = nc.dram_tensor("x", [NPER, C, HW], F32, kind="ExternalInput")
    wt_d = nc.dram_tensor("wt", [C, K], F32, kind="ExternalInput")
    eb_d = nc.dram_tensor("eb", [K], F32, kind="ExternalInput")
    cent_d = nc.dram_tensor("cent", [K, C], F32, kind="ExternalInput")
    out_d = nc.dram_tensor("out", [NPER, K, C], F32, kind="ExternalOutput")

    from contextlib import ExitStack

    with tile.TileContext(nc) as tc:
        with ExitStack() as ctx:
            netvlad_tile_kernel(ctx, tc, out_d, x_d, wt_d, eb_d, cent_d)
    nc.compile()
    return nc


_NC = None


def _get_nc():
    global _NC
    if _NC is None:
        _NC = build_nc()
    return _NC


def make_in_maps(x, conv_w, conv_b, centroids):
    wt = np.ascontiguousarray(conv_w.astype(np.float32).T)  # [C, K]
    eb = np.exp(conv_b.astype(np.float64)).astype(np.float32)  # [K]
    cent = np.ascontiguousarray(centroids.astype(np.float32))
    xs = np.ascontiguousarray(x.astype(np.float32).reshape(NCORES, NPER, C, HW))
    return [
        {"x": xs[c], "wt": wt, "eb": eb, "cent": cent} for c in range(NCORES)
    ]


def kernel(x, conv_w, conv_b, centroids, **_ignored):
    nc = _get_nc()
    in_maps = make_in_maps(x, conv_w, conv_b, centroids)
    res = run_bass_kernel_spmd(nc, in_maps, core_ids=list(range(NCORES)))
    outs = [res.results[c]["out"].reshape(NPER, K * C) for c in range(NCORES)]
    return np.concatenate(outs, axis=0).astype(np.float32)


if __name__ == "__main__":
    nc = build_nc()
    print("built ok:", len(nc.inst_map), "instructions")
